# revision 1
# baseline (speedup 1.0000x reference)
"""GAT layer kernel for Trainium2, 8 NeuronCores, data-parallel over R=b*s.

Self-contained: takes full inputs, returns full output.

v2 design (per core, RC=6 replicas):
  - Projection on PE: h_aug = x_r @ [W(c-major) | Ws | Wd]; h (bf16) plus the
    per-node a_src scalars are written into ONE HBM row per node:
    row n = [h r0..r5 (6*256 bf16, c-major) | a_src 24 bf16 | pad] (3328 B).
    a_dst scalars stay in SBUF ([125, nt, 24]).
  - Edge phase is chunked by dst-tile (125 dsts, all its edges, dst-sorted,
    padded to 128-slot tiles). Per chunk ONE dma_gather fetches, for every
    edge slot, the full row of its src node (~1800 descriptors, under the
    SWDGE FIFO limit).
  - z = a_src[src] (gathered) + a_dst[dst] (PE expand via transposed one-hot)
    p = exp(leaky_relu(z)) ; den = segment_sum(p) (PE one-hot);
    denrec = 0.25/den (node space - applied after aggregation, so no
    alpha normalization per edge is needed).
  - msg = hg * p (DVE, bf16, heads broadcast over c-major layout)
    num = segment_sum(msg) (PE one-hot, f32 PSUM, all 6 replicas per chunk)
  - out = sum_h denrec[d,h]*num[d,(c,h)] + bias  (node space, then DMA out).
"""

import math
import numpy as np
import ml_dtypes

B, S, N, F = 4, 12, 1000, 64
H, C = 4, 64
HC = H * C            # 256
R = B * S             # 48
NCORES = 8
RC = R // NCORES      # 6 replicas per core
NEG_SLOPE = 0.2
DTW = 125             # dst-tile width (8 tiles cover N=1000)
NDT = N // DTW        # 8
AC = RC * H           # 24 active scalar columns
ROWW = 1664           # h_hbm row width in bf16 (6*256 h + 24 as + pad) = 3328B

_CACHE = {}


# --------------------------------------------------------------------------
# host-side index preprocessing
# --------------------------------------------------------------------------
def _prep_edges(edge_index):
    src0 = np.asarray(edge_index[0], dtype=np.int64)
    dst0 = np.asarray(edge_index[1], dtype=np.int64)
    keep = src0 != dst0                      # PyG remove_self_loops + NEG_INF mask
    s_all = np.concatenate([src0[keep], np.arange(N, dtype=np.int64)])
    d_all = np.concatenate([dst0[keep], np.arange(N, dtype=np.int64)])
    order = np.argsort(d_all, kind="stable")
    s_all, d_all = s_all[order], d_all[order]

    # per dst-tile slot lists, each padded to a multiple of 128
    chunks = []
    for dt in range(NDT):
        lo, hi = dt * DTW, (dt + 1) * DTW
        m = (d_all >= lo) & (d_all < hi)
        ss, dd = s_all[m], d_all[m]
        cnt = len(ss)
        ntile = max(1, math.ceil(cnt / 128))
        pad = ntile * 128 - cnt
        ss = np.concatenate([ss, np.full(pad, 1000, np.int64)])   # pad -> row 1000
        dd = np.concatenate([dd, np.full(pad, lo, np.int64)])
        real = np.concatenate([np.ones(cnt, bool), np.zeros(pad, bool)])
        # one-hot [p, t, dlocal] and transposed [t, dlocal, p]
        oh = np.zeros((128, ntile, DTW), np.float32)
        for j in range(ntile * 128):
            if real[j]:
                oh[j % 128, j // 128, dd[j] - lo] = 1.0
        chunks.append(dict(ntile=ntile, src=ss, oh=oh.astype(ml_dtypes.bfloat16),
                           ohT=np.ascontiguousarray(
                               oh.transpose(2, 1, 0)).astype(ml_dtypes.bfloat16)))

    maxt = max(c["ntile"] for c in chunks)
    T = sum(c["ntile"] for c in chunks)
    # index tensor: per chunk, slots wrapped [16, slots/16], replicated to 128
    ihw = np.zeros((128, T * 8), np.int16)   # 128 slots = 8 idx columns
    oh_all = np.zeros((128, T, DTW), ml_dtypes.bfloat16)
    ohT_all = np.zeros((128, T, 128), ml_dtypes.bfloat16)
    t0 = 0
    for c in chunks:
        nt_, ss = c["ntile"], c["src"]
        ni = nt_ * 128
        a = np.zeros((16, ni // 16), np.int16)
        a[np.arange(ni) % 16, np.arange(ni) // 16] = ss.astype(np.int16)
        ihw[:, t0 * 8:(t0 + nt_) * 8] = np.tile(a, (8, 1))
        oh_all[:, t0:t0 + nt_, :] = c["oh"]
        ohT_all[:DTW, t0:t0 + nt_, :] = c["ohT"].transpose(0, 1, 2).reshape(
            DTW, nt_, 128)
        t0 += nt_
    tile_of = np.concatenate([[i] * c["ntile"] for i, c in enumerate(chunks)])
    return {
        "T": T, "maxt": maxt, "ntiles": [c["ntile"] for c in chunks],
        "oh": np.ascontiguousarray(oh_all.reshape(128, T * DTW)),
        "ohT": np.ascontiguousarray(ohT_all.reshape(128, T * 128)),
        "ih": ihw, "tile_of": tile_of,
    }


def _prep_weights(W, att_src, att_dst):
    W = np.asarray(W, np.float32)
    Ws = np.zeros((F, H), np.float32)
    Wd = np.zeros((F, H), np.float32)
    for h in range(H):
        Ws[:, h] = W[:, h * C:(h + 1) * C] @ np.asarray(att_src, np.float32)[h]
        Wd[:, h] = W[:, h * C:(h + 1) * C] @ np.asarray(att_dst, np.float32)[h]
    # c-major head interleave: device col c*4+h = W col h*64+c
    Wc = np.empty_like(W)
    for h in range(H):
        Wc[:, np.arange(C) * H + h] = W[:, h * C:(h + 1) * C]
    return np.concatenate([Wc, Ws, Wd], axis=1)      # [64, 264]


def _make_in_maps(x, W, att_src, att_dst, bias, ed):
    waug = _prep_weights(W, att_src, att_dst)
    bias_slab = np.tile(np.asarray(bias, np.float32)[None, :],
                        (128, RC)).reshape(128, RC * F)
    xr = np.ascontiguousarray(np.asarray(x, np.float32)).reshape(R, N, F)
    in_maps = []
    for cidx in range(NCORES):
        xc = xr[cidx * RC:(cidx + 1) * RC]
        xT = np.ascontiguousarray(xc.transpose(2, 0, 1).reshape(F, RC * N))
        in_maps.append({
            "xT": xT, "w_aug": waug, "oh": ed["oh"], "ohT": ed["ohT"],
            "ih": ed["ih"], "bias_slab": bias_slab,
        })
    return in_maps


# --------------------------------------------------------------------------
# device program
# --------------------------------------------------------------------------
def _build_program(ed):
    import concourse.bass as bass
    import concourse.mybir as mybir
    import concourse.tile as tile
    from concourse import bacc

    T, maxt = ed["T"], ed["maxt"]
    ntiles = ed["ntiles"]
    f32 = mybir.dt.float32
    bf16 = mybir.dt.bfloat16
    i16 = mybir.dt.int16
    Alu = mybir.AluOpType
    Act = mybir.ActivationFunctionType

    nc = bacc.Bacc("TRN2", target_bir_lowering=False, debug=False,
                   enable_asserts=False, num_devices=NCORES)

    xT_d = nc.dram_tensor("xT", [F, RC * N], f32, kind="ExternalInput").ap()
    waug_d = nc.dram_tensor("w_aug", [F, 264], f32, kind="ExternalInput").ap()
    oh_d = nc.dram_tensor("oh", [128, T * DTW], bf16, kind="ExternalInput").ap()
    ohT_d = nc.dram_tensor("ohT", [128, T * 128], bf16, kind="ExternalInput").ap()
    ih_d = nc.dram_tensor("ih", [128, T * 8], i16, kind="ExternalInput").ap()
    bias_d = nc.dram_tensor("bias_slab", [128, RC * F], f32, kind="ExternalInput").ap()
    out_d = nc.dram_tensor("out", [RC, N, F], f32, kind="ExternalOutput").ap()

    with tile.TileContext(nc) as tc:
        with (
            tc.tile_pool(name="const", bufs=1) as constp,
            tc.tile_pool(name="dram", bufs=1, space="DRAM") as dramp,
            tc.tile_pool(name="stage", bufs=3) as stagep,
            tc.tile_pool(name="edge", bufs=2) as edgep,
            tc.tile_pool(name="big", bufs=2) as bigp,
            tc.tile_pool(name="fin", bufs=2) as finp,
            tc.tile_pool(name="ppsum", bufs=3, space="PSUM") as ppsum,
            tc.tile_pool(name="npsum", bufs=2, space="PSUM") as npsum,
        ):
            h_hbm = dramp.tile([N + 1, ROWW], bf16)

            # ---- constants ----
            waug = constp.tile([F, 264], f32)
            nc.sync.dma_start(waug[:], waug_d)
            oh = constp.tile([128, T, DTW], bf16)
            nc.sync.dma_start(oh[:], oh_d.rearrange("p (t d) -> p t d", d=DTW))
            ohT = constp.tile([128, T, 128], bf16)
            nc.sync.dma_start(ohT[:], ohT_d.rearrange("p (t e) -> p t e", e=128))
            ih = constp.tile([128, T * 8], i16)
            nc.sync.dma_start(ih[:], ih_d)
            bias_sl = constp.tile([128, RC, F], f32)
            nc.sync.dma_start(bias_sl[:], bias_d.rearrange("p (r f) -> p r f", f=F))

            # pad row 1000: h-part zeros, as-part -1000 => p == 0 for pad slots
            padrow = constp.tile([1, ROWW], bf16)
            nc.vector.memset(padrow[:], 0.0)
            nc.vector.memset(padrow[:, RC * HC:RC * HC + AC], -1000.0)
            nc.sync.dma_start(h_hbm[N:N + 1, :], padrow[:])

            # ---- phase A: projection; fills h_hbm, ad_sb, as_sb ----
            ad_sb = constp.tile([DTW, NDT, AC], bf16)
            as_sb = constp.tile([DTW, NDT, AC], bf16)
            for r in range(RC):
                xt = stagep.tile([F, N], f32, tag="xt")
                nc.sync.dma_start(xt[:], xT_d[:, r * N:(r + 1) * N])
                hslab = stagep.tile([DTW, NDT, HC], bf16, tag="hslab")
                for nt in range(NDT):
                    n0 = nt * DTW
                    ps = ppsum.tile([DTW, 264], f32, tag="scratch")
                    nc.tensor.matmul(out=ps[:], lhsT=xt[:, n0:n0 + DTW],
                                     rhs=waug[:], start=True, stop=True)
                    nc.scalar.copy(out=hslab[:, nt, :], in_=ps[:, 0:HC])
                    nc.vector.tensor_copy(out=as_sb[:, nt, 4 * r:4 * r + 4],
                                          in_=ps[:, HC:HC + 4])
                    nc.vector.tensor_copy(out=ad_sb[:, nt, 4 * r:4 * r + 4],
                                          in_=ps[:, HC + 4:HC + 8])
                nc.sync.dma_start(
                    h_hbm[0:N, r * HC:(r + 1) * HC].rearrange(
                        "(a d) e -> d a e", d=DTW), hslab[:])
            nc.sync.dma_start(
                h_hbm[0:N, RC * HC:RC * HC + AC].rearrange(
                    "(a d) e -> d a e", d=DTW), as_sb[:])

            # ---- per dst-tile chunks ----
            t0 = 0
            for dt in range(NDT):
                nt_ = ntiles[dt]
                ni = nt_ * 128
                assert ni <= 2032, "gather exceeds SWDGE FIFO; split needed"
                hg = bigp.tile([128, maxt, ROWW], bf16, tag="big")
                nc.gpsimd.dma_gather(
                    out_ap=hg[:, 0:nt_, :], in_ap=h_hbm[:],
                    idxs_ap=ih[:, t0 * 8:(t0 + nt_) * 8],
                    num_idxs=ni, num_idxs_reg=ni, elem_size=ROWW,
                    single_packet=False)

                # ad expand: [128(e), nt_*24] psum via transposed one-hot
                eps = ppsum.tile([128, maxt, AC], f32, tag="scratch", name="eps")
                for t in range(nt_):
                    nc.tensor.matmul(out=eps[:, t, :], lhsT=ohT[0:DTW, t0 + t, :],
                                     rhs=ad_sb[:, dt, :], start=True, stop=True)
                # z = as + ad ; leaky relu ; exp
                z = edgep.tile([128, maxt, AC], f32, tag="z")
                nc.vector.tensor_tensor(
                    out=z[:, 0:nt_, :],
                    in0=hg[:, 0:nt_, RC * HC:RC * HC + AC],
                    in1=eps[:, 0:nt_, :], op=Alu.add)
                nc.vector.scalar_tensor_tensor(
                    out=z[:, 0:nt_, :], in0=z[:, 0:nt_, :], scalar=NEG_SLOPE,
                    in1=z[:, 0:nt_, :], op0=Alu.mult, op1=Alu.max)
                p_bf = edgep.tile([128, maxt, AC], bf16, tag="p")
                nc.scalar.activation(out=p_bf[:, 0:nt_, :], in_=z[:, 0:nt_, :],
                                     func=Act.Exp)

                # den = segsum(p); denrec = 0.25/den
                den_ps = ppsum.tile([DTW, AC], f32, tag="scratch", name="den_ps")
                for t in range(nt_):
                    nc.tensor.matmul(out=den_ps[:], lhsT=oh[:, t0 + t, :],
                                     rhs=p_bf[:, t, :],
                                     start=(t == 0), stop=(t == nt_ - 1))
                denrec = stagep.tile([DTW, AC], f32, tag="denrec")
                nc.vector.reciprocal(out=denrec[:], in_=den_ps[:])
                nc.vector.tensor_scalar_mul(denrec[:], denrec[:], 0.25)

                HW2 = RC * HC // 2          # 768 cols per half (3 replicas)
                for half in range(2):
                    c0 = half * HW2
                    for r in range(3 * half, 3 * half + 3):
                        hgr = hg[:, 0:nt_, r * HC:(r + 1) * HC].rearrange(
                            "p t (c h) -> p t c h", h=H)
                        pb = p_bf[:, 0:nt_, 4 * r:4 * r + 4].rearrange(
                            "p t (o h) -> p t o h", o=1).to_broadcast(
                            [128, nt_, C, H])
                        nc.vector.tensor_tensor(out=hgr, in0=hgr, in1=pb,
                                                op=Alu.mult)
                    nps = npsum.tile([DTW, RC // 2, HC], f32, tag="num")
                    npsf = nps[:].rearrange("d r e -> d (r e)")
                    for t in range(nt_):
                        nc.tensor.matmul(out=npsf[:, 0:512],
                                         lhsT=oh[:, t0 + t, :],
                                         rhs=hg[:, t, c0:c0 + 512],
                                         start=(t == 0), stop=(t == nt_ - 1))
                    for t in range(nt_):
                        nc.tensor.matmul(out=npsf[:, 512:768],
                                         lhsT=oh[:, t0 + t, :],
                                         rhs=hg[:, t, c0 + 512:c0 + 768],
                                         start=(t == 0), stop=(t == nt_ - 1))

                    # finalize: numn = num * denrec, head-sum, bias, DMA out
                    numn = finp.tile([DTW, RC // 2, HC], f32, tag="numn")
                    drb = denrec[:, half * 12:half * 12 + 12].rearrange(
                        "d (r o h) -> d r o h", h=H, o=1).to_broadcast(
                        [DTW, RC // 2, C, H])
                    nc.vector.tensor_tensor(
                        out=numn[:].rearrange("d r (c h) -> d r c h", h=H),
                        in0=nps[:].rearrange("d r (c h) -> d r c h", h=H),
                        in1=drb, op=Alu.mult)
                    n4 = numn[:].rearrange("d r (c h) -> d r c h", h=H)
                    t1 = finp.tile([DTW, RC // 2, C], f32, tag="t1")
                    t2 = finp.tile([DTW, RC // 2, C], f32, tag="t2")
                    ob = finp.tile([DTW, RC // 2, C], f32, tag="ob")
                    nc.vector.tensor_tensor(out=t1[:], in0=n4[:, :, :, 0],
                                            in1=n4[:, :, :, 1], op=Alu.add)
                    nc.vector.tensor_tensor(out=t2[:], in0=n4[:, :, :, 2],
                                            in1=n4[:, :, :, 3], op=Alu.add)
                    nc.vector.tensor_tensor(out=t1[:], in0=t1[:], in1=t2[:],
                                            op=Alu.add)
                    nc.vector.tensor_tensor(
                        out=ob[:], in0=t1[:],
                        in1=bias_sl[0:DTW, half * 3:half * 3 + 3, :], op=Alu.add)
                    nc.sync.dma_start(
                        out_d[half * 3:half * 3 + 3,
                              dt * DTW:(dt + 1) * DTW, :].rearrange(
                            "r d f -> d r f"), ob[:])
                t0 += nt_

    nc.compile()
    return nc


# --------------------------------------------------------------------------
# public entry point
# --------------------------------------------------------------------------
def kernel(x, edge_index, W, att_src, att_dst, bias):
    key = hash(np.asarray(edge_index).tobytes())
    if key not in _CACHE:
        ed = _prep_edges(edge_index)
        _CACHE[key] = (_build_program(ed), ed)
    nc, ed = _CACHE[key]

    in_maps = _make_in_maps(x, W, att_src, att_dst, bias, ed)
    from concourse import bass_utils
    res = bass_utils.run_bass_kernel_spmd(nc, in_maps, core_ids=list(range(NCORES)))
    outs = [res.results[c]["out"] for c in range(NCORES)]
    out = np.concatenate(outs, axis=0).reshape(B, S, N, F).astype(np.float32)
    return out



# revision 12
# speedup vs baseline: 1.1112x; 1.1112x over previous
"""GAT layer kernel for Trainium2, 8 NeuronCores, data-parallel over R=b*s.

Self-contained: takes full inputs, returns full output.

v3 design (per core, RC=6 replicas):
  - Projection on PE in bf16 (x, W host-cast); the 1/4 head-mean factor is
    folded into the projection weights (h is stored pre-scaled by 0.25 while
    the att_src/att_dst columns use the unscaled W, so attention is exact).
  - h (bf16) plus per-node a_src scalars are written into ONE HBM row per
    node: row n = [h r0..r5 (6*256 bf16, c-major) | a_src 24 bf16 | pad]
    (3328 B).  a_dst scalars stay in SBUF.
  - Edge phase is chunked by dst-tile (125 dsts, dst-sorted slots padded to
    128-slot tiles).  Per chunk ONE dma_gather fetches each edge slot's src
    row.  Descriptor generation (~13.4us/chunk on the Pool engine) is
    front-loaded with prepare_only=True across 4 SWDGE queues so it runs
    during the projection; trigger_dma fires each transfer once h is in HBM.
  - z = a_src[src] (gathered) + a_dst[dst] (PE expand via transposed one-hot)
    p = exp(leaky_relu(z)); den = segment_sum(p) via PE one-hot right after
    exp (so the reciprocal overlaps the big matmuls).
  - msg = hg * p (DVE bf16 4x mode, heads broadcast over c-major layout)
    num = segment_sum(msg) (PE one-hot, f32 PSUM, 3x512-col matmuls/tile).
  - finalize: one DVE pass over PSUM (num * (1/den) -> bf16), head-sum in
    bf16, + bias, DMA out.  num is pre-scaled by 0.25 so out = sum_h
    num_h/den_h + bias directly.
"""

import math
import numpy as np
import ml_dtypes

B, S, N, F = 4, 12, 1000, 64
H, C = 4, 64
HC = H * C            # 256
R = B * S             # 48
NCORES = 8
RC = R // NCORES      # 6 replicas per core
NEG_SLOPE = 0.2
DTW = 125             # dst-tile width (8 tiles cover N=1000)
NDT = N // DTW        # 8
AC = RC * H           # 24 active scalar columns
ROWW = 1664           # h_hbm row width in bf16 (6*256 h + 24 as + pad) = 3328B
NQ = 4                # SWDGE queues

_CACHE = {}


# --------------------------------------------------------------------------
# host-side index preprocessing
# --------------------------------------------------------------------------
def _prep_edges(edge_index):
    src0 = np.asarray(edge_index[0], dtype=np.int64)
    dst0 = np.asarray(edge_index[1], dtype=np.int64)
    keep = src0 != dst0                      # PyG remove_self_loops + NEG_INF mask
    s_all = np.concatenate([src0[keep], np.arange(N, dtype=np.int64)])
    d_all = np.concatenate([dst0[keep], np.arange(N, dtype=np.int64)])
    order = np.argsort(d_all, kind="stable")
    s_all, d_all = s_all[order], d_all[order]

    # per dst-tile slot lists, each padded to a multiple of 128
    chunks = []
    for dt in range(NDT):
        lo, hi = dt * DTW, (dt + 1) * DTW
        m = (d_all >= lo) & (d_all < hi)
        ss, dd = s_all[m], d_all[m]
        cnt = len(ss)
        ntile = max(1, math.ceil(cnt / 128))
        pad = ntile * 128 - cnt
        ss = np.concatenate([ss, np.full(pad, 1000, np.int64)])   # pad -> row 1000
        dd = np.concatenate([dd, np.full(pad, lo, np.int64)])
        real = np.concatenate([np.ones(cnt, bool), np.zeros(pad, bool)])
        # one-hot [p, t, dlocal] (slot j = t*128 + p)
        oh = np.zeros((128, ntile, DTW), np.float32)
        for j in range(ntile * 128):
            if real[j]:
                oh[j % 128, j // 128, dd[j] - lo] = 1.0
        chunks.append(dict(ntile=ntile, src=ss, oh=oh.astype(ml_dtypes.bfloat16),
                           ohT=np.ascontiguousarray(
                               oh.transpose(2, 1, 0)).astype(ml_dtypes.bfloat16)))

    maxt = max(c["ntile"] for c in chunks)
    T = sum(c["ntile"] for c in chunks)
    # index tensor: per chunk, slots wrapped [16, slots/16], replicated to 128
    ihw = np.zeros((128, T * 8), np.int16)   # 128 slots = 8 idx columns
    oh_all = np.zeros((128, T, DTW), ml_dtypes.bfloat16)
    ohT_all = np.zeros((128, T, 128), ml_dtypes.bfloat16)
    t0 = 0
    for c in chunks:
        nt_, ss = c["ntile"], c["src"]
        ni = nt_ * 128
        a = np.zeros((16, ni // 16), np.int16)
        a[np.arange(ni) % 16, np.arange(ni) // 16] = ss.astype(np.int16)
        ihw[:, t0 * 8:(t0 + nt_) * 8] = np.tile(a, (8, 1))
        oh_all[:, t0:t0 + nt_, :] = c["oh"]
        ohT_all[:DTW, t0:t0 + nt_, :] = c["ohT"].reshape(DTW, nt_, 128)
        t0 += nt_
    return {
        "T": T, "maxt": maxt, "ntiles": [c["ntile"] for c in chunks],
        "oh": np.ascontiguousarray(oh_all.reshape(128, T * DTW)),
        "ohT": np.ascontiguousarray(ohT_all.reshape(128, T * 128)),
        "ih": ihw,
    }


def _prep_weights(W, att_src, att_dst):
    W = np.asarray(W, np.float32)
    Ws = np.zeros((F, H), np.float32)
    Wd = np.zeros((F, H), np.float32)
    for h in range(H):
        Ws[:, h] = W[:, h * C:(h + 1) * C] @ np.asarray(att_src, np.float32)[h]
        Wd[:, h] = W[:, h * C:(h + 1) * C] @ np.asarray(att_dst, np.float32)[h]
    # c-major head interleave: device col c*4+h = W col h*64+c
    Wc = np.empty_like(W)
    for h in range(H):
        Wc[:, np.arange(C) * H + h] = W[:, h * C:(h + 1) * C]
    # fold the 1/H head-mean into h (att columns stay unscaled -> z exact)
    waug = np.concatenate([0.25 * Wc, Ws, Wd], axis=1)      # [64, 264]
    return waug.astype(ml_dtypes.bfloat16)


def _make_in_maps(x, W, att_src, att_dst, bias, ed):
    waug = _prep_weights(W, att_src, att_dst)
    bias_slab = np.tile(np.asarray(bias, np.float32)[None, :],
                        (128, RC)).reshape(128, RC * F)
    xr = np.ascontiguousarray(np.asarray(x, np.float32)).reshape(R, N, F)
    in_maps = []
    for cidx in range(NCORES):
        xc = xr[cidx * RC:(cidx + 1) * RC]
        xT = np.ascontiguousarray(xc.transpose(2, 0, 1).reshape(F, RC * N)
                                  ).astype(ml_dtypes.bfloat16)
        in_maps.append({
            "xT": xT, "w_aug": waug, "oh": ed["oh"], "ohT": ed["ohT"],
            "ih": ed["ih"], "bias_slab": bias_slab,
        })
    return in_maps


# --------------------------------------------------------------------------
# device program
# --------------------------------------------------------------------------
def _build_program(ed):
    import concourse.bass as bass
    import concourse.mybir as mybir
    import concourse.tile as tile
    from concourse import bacc

    T, maxt = ed["T"], ed["maxt"]
    ntiles = ed["ntiles"]
    f32 = mybir.dt.float32
    bf16 = mybir.dt.bfloat16
    i16 = mybir.dt.int16
    Alu = mybir.AluOpType
    Act = mybir.ActivationFunctionType

    nc = bacc.Bacc("TRN2", target_bir_lowering=False, debug=False,
                   enable_asserts=False, num_devices=NCORES,
                   num_swdge_queues=NQ)

    xT_d = nc.dram_tensor("xT", [F, RC * N], bf16, kind="ExternalInput").ap()
    waug_d = nc.dram_tensor("w_aug", [F, 264], bf16, kind="ExternalInput").ap()
    oh_d = nc.dram_tensor("oh", [128, T * DTW], bf16, kind="ExternalInput").ap()
    ohT_d = nc.dram_tensor("ohT", [128, T * 128], bf16, kind="ExternalInput").ap()
    ih_d = nc.dram_tensor("ih", [128, T * 8], i16, kind="ExternalInput").ap()
    bias_d = nc.dram_tensor("bias_slab", [128, RC * F], f32, kind="ExternalInput").ap()
    out_d = nc.dram_tensor("out", [RC, N, F], f32, kind="ExternalOutput").ap()

    # chunk tile offsets
    t0s = []
    acc = 0
    for nt_ in ntiles:
        t0s.append(acc)
        acc += nt_

    with tile.TileContext(nc) as tc:
        with (
            tc.tile_pool(name="const", bufs=1) as constp,
            tc.tile_pool(name="dram", bufs=1, space="DRAM") as dramp,
            # Edge-phase SBUF pools opened BEFORE phase A so the deferred
            # gather writes (prepare_only) never alias phase-A tiles -- Tile
            # does not create WAR edges for deferred prep writes across pool
            # reuse.
            tc.tile_pool(name="hgp", bufs=3) as hgp,
            tc.tile_pool(name="ohp", bufs=2) as ohp,
            tc.tile_pool(name="ohTp", bufs=2) as ohTp,
            tc.tile_pool(name="edge", bufs=2) as edgep,
            tc.tile_pool(name="fin", bufs=2) as finp,
        ):
            h_hbm = dramp.tile([N + 1, ROWW], bf16)

            # ---- small constants (ih first: descgen depends on it) ----
            ih = constp.tile([128, T * 8], i16)
            nc.sync.dma_start(ih[:], ih_d)
            waug = constp.tile([F, 264], bf16)
            nc.sync.dma_start(waug[:], waug_d)
            bias_sl = constp.tile([128, RC, F], f32)
            nc.sync.dma_start(bias_sl[:], bias_d.rearrange("p (r f) -> p r f", f=F))

            # a_src / a_dst node-space scalars: [125, NDT, RC, 8]
            sc_sb = constp.tile([DTW, NDT, RC, 8], bf16)

            # pad row 1000: h-part zeros, as-part -1000 => p == 0 for pad slots
            padrow = constp.tile([1, ROWW], bf16)
            nc.vector.memset(padrow[:], 0.0)
            nc.vector.memset(padrow[:, RC * HC:RC * HC + AC], -1000.0)
            nc.sync.dma_start(h_hbm[N:N + 1, :], padrow[:])

            # ---- phase A: projection; fills h_hbm + sc_sb ----
            with (
                tc.tile_pool(name="stage", bufs=2) as stagep,
                tc.tile_pool(name="ppsum", bufs=3, space="PSUM") as ppsum,
            ):
                for r in range(RC):
                    xt = stagep.tile([F, N], bf16, tag="xt")
                    nc.sync.dma_start(xt[:], xT_d[:, r * N:(r + 1) * N])
                    hslab = stagep.tile([DTW, NDT, HC], bf16, tag="hslab")
                    for nt in range(NDT):
                        n0 = nt * DTW
                        ps = ppsum.tile([DTW, 264], f32, tag="scratch")
                        nc.tensor.matmul(out=ps[:], lhsT=xt[:, n0:n0 + DTW],
                                         rhs=waug[:], start=True, stop=True)
                        # split PSUM->SBUF copies across ACT+DVE so phase A
                        # is not serialized on either (GpSimd cannot read PSUM)
                        eng = (nc.scalar.copy, nc.vector.tensor_copy)[nt % 2]
                        eng(out=hslab[:, nt, :], in_=ps[:, 0:HC])
                        nc.vector.tensor_copy(out=sc_sb[:, nt, r, :],
                                              in_=ps[:, HC:HC + 8])
                    nc.sync.dma_start(
                        h_hbm[0:N, r * HC:(r + 1) * HC].rearrange(
                            "(a d) e -> d a e", d=DTW), hslab[:])
                # a_src columns of every row (8 DMAs: DMA APs max 3 dims)
                for a in range(NDT):
                    nc.sync.dma_start(
                        h_hbm[a * DTW:(a + 1) * DTW,
                              RC * HC:RC * HC + AC].rearrange(
                            "d (r e) -> d r e", e=4),
                        sc_sb[:, a, :, 0:4])

            # ---- edge phase ----
            with (
                tc.tile_pool(name="npsum", bufs=2, space="PSUM") as npsum,
                tc.tile_pool(name="epsum", bufs=1, space="PSUM") as epsum,
                tc.tile_pool(name="dpsum", bufs=1, space="PSUM") as dpsum,
            ):
                for dt in range(NDT):
                    nt_ = ntiles[dt]
                    t0 = t0s[dt]
                    ni = nt_ * 128
                    assert ni <= 2032, "gather exceeds SWDGE FIFO"
                    hg = hgp.tile([128, maxt, ROWW], bf16, tag="hg")
                    nc.gpsimd.dma_gather(
                        out_ap=hg[:, 0:nt_, :], in_ap=h_hbm[:],
                        idxs_ap=ih[:, t0 * 8:(t0 + nt_) * 8],
                        num_idxs=ni, num_idxs_reg=ni, elem_size=ROWW,
                        single_packet=False, queue_num=dt % NQ)

                    ohc = ohp.tile([128, maxt, DTW], bf16, tag="oh")
                    nc.sync.dma_start(
                        ohc[:, 0:nt_, :],
                        oh_d[:, t0 * DTW:(t0 + nt_) * DTW].rearrange(
                            "p (t d) -> p t d", d=DTW))
                    ohTc = ohTp.tile([128, maxt, 128], bf16, tag="ohT")
                    nc.sync.dma_start(
                        ohTc[:, 0:nt_, :],
                        ohT_d[:, t0 * 128:(t0 + nt_) * 128].rearrange(
                            "p (t e) -> p t e", e=128))

                    # ad expand: [128(e), nt_, 24] psum via transposed one-hot
                    eps = epsum.tile([128, maxt, AC], f32, tag="eps")
                    for t in range(nt_):
                        nc.tensor.matmul(out=eps[:, t, :],
                                         lhsT=ohTc[0:DTW, t, :],
                                         rhs=sc_sb[:, dt, :, 4:8],
                                         start=True, stop=True)
                    # z = as + ad ; leaky relu ; exp
                    z = edgep.tile([128, maxt, AC], f32, tag="z")
                    nc.vector.tensor_tensor(
                        out=z[:, 0:nt_, :],
                        in0=hg[:, 0:nt_, RC * HC:RC * HC + AC],
                        in1=eps[:, 0:nt_, :], op=Alu.add)
                    nc.vector.scalar_tensor_tensor(
                        out=z[:, 0:nt_, :], in0=z[:, 0:nt_, :], scalar=NEG_SLOPE,
                        in1=z[:, 0:nt_, :], op0=Alu.mult, op1=Alu.max)
                    p_bf = edgep.tile([128, maxt, AC], bf16, tag="p")
                    nc.scalar.activation(out=p_bf[:, 0:nt_, :], in_=z[:, 0:nt_, :],
                                         func=Act.Exp)

                    # den = segsum(p) -- early, so reciprocal overlaps num mms
                    den_ps = dpsum.tile([DTW, AC], f32, tag="den")
                    for t in range(nt_):
                        nc.tensor.matmul(out=den_ps[:], lhsT=ohc[:, t, :],
                                         rhs=p_bf[:, t, :],
                                         start=(t == 0), stop=(t == nt_ - 1))
                    drb = finp.tile([DTW, AC], f32, tag="drb")
                    nc.vector.reciprocal(out=drb[:], in_=den_ps[:])

                    # msg = hg * p (in-place, bf16 4x DVE)
                    for r in range(RC):
                        hgr = hg[:, 0:nt_, r * HC:(r + 1) * HC].rearrange(
                            "p t (c h) -> p t c h", h=H)
                        pb = p_bf[:, 0:nt_, 4 * r:4 * r + 4].rearrange(
                            "p t (o h) -> p t o h", o=1).to_broadcast(
                            [128, nt_, C, H])
                        nc.vector.tensor_tensor(out=hgr, in0=hgr, in1=pb,
                                                op=Alu.mult)

                    # num = segsum(msg): 3x512-col accumulating matmuls per tile
                    nps = npsum.tile([DTW, RC * HC], f32, tag="num")
                    for t in range(nt_):
                        for cb in range(3):
                            nc.tensor.matmul(
                                out=nps[:, cb * 512:(cb + 1) * 512],
                                lhsT=ohc[:, t, :],
                                rhs=hg[:, t, cb * 512:(cb + 1) * 512],
                                start=(t == 0), stop=(t == nt_ - 1))

                    # finalize: numn = num * (1/den) [single PSUM pass],
                    # head-sum, + bias, DMA out
                    numn = finp.tile([DTW, RC, C, H], bf16, tag="numn")
                    drbb = drb[:].rearrange("d (r o h) -> d r o h", h=H, o=1
                                            ).to_broadcast([DTW, RC, C, H])
                    nc.vector.tensor_tensor(
                        out=numn[:],
                        in0=nps[:].rearrange("d (r c h) -> d r c h", h=H, c=C),
                        in1=drbb, op=Alu.mult)
                    t12 = finp.tile([DTW, RC, C], bf16, tag="t12")
                    t34 = finp.tile([DTW, RC, C], bf16, tag="t34")
                    ob = finp.tile([DTW, RC, C], f32, tag="ob")
                    nc.vector.tensor_tensor(out=t12[:], in0=numn[:, :, :, 0],
                                            in1=numn[:, :, :, 1], op=Alu.add)
                    nc.vector.tensor_tensor(out=t34[:], in0=numn[:, :, :, 2],
                                            in1=numn[:, :, :, 3], op=Alu.add)
                    nc.vector.tensor_tensor(out=ob[:], in0=t12[:], in1=t34[:],
                                            op=Alu.add)
                    nc.vector.tensor_tensor(out=ob[:], in0=ob[:],
                                            in1=bias_sl[0:DTW, :, :], op=Alu.add)
                    nc.sync.dma_start(
                        out_d[:, dt * DTW:(dt + 1) * DTW, :].rearrange(
                            "r d f -> d r f"), ob[:])

    nc.compile()
    return nc


# --------------------------------------------------------------------------
# public entry point
# --------------------------------------------------------------------------
def kernel(x, edge_index, W, att_src, att_dst, bias):
    key = hash(np.asarray(edge_index).tobytes())
    if key not in _CACHE:
        ed = _prep_edges(edge_index)
        _CACHE[key] = (_build_program(ed), ed)
    nc, ed = _CACHE[key]

    in_maps = _make_in_maps(x, W, att_src, att_dst, bias, ed)
    from concourse import bass_utils
    res = bass_utils.run_bass_kernel_spmd(nc, in_maps, core_ids=list(range(NCORES)))
    outs = [res.results[c]["out"] for c in range(NCORES)]
    out = np.concatenate(outs, axis=0).reshape(B, S, N, F).astype(np.float32)
    return out


# revision 14
# speedup vs baseline: 1.1814x; 1.0632x over previous
"""GAT layer kernel for Trainium2, 8 NeuronCores, data-parallel over R=b*s.

Self-contained: takes full inputs, returns full output.

v3.1 design (per core, RC=6 replicas):
  - Projection on PE in bf16 (x, W host-cast); the 1/4 head-mean factor is
    folded into the projection weights.  6-deep PSUM pipeline keeps the PE
    at full clock; PSUM->SBUF copies alternate ACT/DVE.
  - h (bf16) plus per-node a_src scalars are written into ONE HBM row per
    node: row n = [h r0..r5 (6*256 bf16, c-major) | a_src 24 bf16 | pad]
    (3328 B).  a_dst scalars stay in SBUF (r-contiguous).
  - Edge phase chunked by dst-tile (125 dsts, dst-sorted slots padded to
    128-slot tiles).  Per chunk ONE dma_gather on a SINGLE SWDGE queue
    (sequential completion = pipelined compute).  Descriptor generation is
    front-loaded with prepare_only (it only needs the index tensor), each
    transfer fired by trigger_dma once h is in HBM.  Data-readiness for
    consumers is enforced by explicit per-engine wait_ge on the gather
    semaphore (Tile's automatic consumer waits fire at descgen completion,
    not DMA completion, for deferred preps).
  - z = a_src[src] (gathered) + a_dst[dst] (PE expand via transposed one-hot)
    p = exp(leaky_relu(z)); den = segment_sum(p) (PE one-hot) right after exp
    so the reciprocal overlaps the num matmuls.
  - msg = hg * p (DVE bf16, heads broadcast over c-major layout)
    num = segment_sum(msg): 3x512-col accumulating matmuls per tile into a
    single 4-bank PSUM tile that also holds den and the eps scratch.
  - finalize: one DVE pass over PSUM (num * (1/den) -> bf16 h-major), 4x-mode
    head-sums, + bias, DMA out.
"""

import math
import numpy as np
import ml_dtypes

B, S, N, F = 4, 12, 1000, 64
H, C = 4, 64
HC = H * C            # 256
R = B * S             # 48
NCORES = 8
RC = R // NCORES      # 6 replicas per core
NEG_SLOPE = 0.2
DTW = 125             # dst-tile width (8 tiles cover N=1000)
NDT = N // DTW        # 8
AC = RC * H           # 24 active scalar columns
ROWW = 1664           # h_hbm row width in bf16 (6*256 h + 24 as + pad) = 3328B

_CACHE = {}


# --------------------------------------------------------------------------
# host-side index preprocessing
# --------------------------------------------------------------------------
def _prep_edges(edge_index):
    src0 = np.asarray(edge_index[0], dtype=np.int64)
    dst0 = np.asarray(edge_index[1], dtype=np.int64)
    keep = src0 != dst0                      # PyG remove_self_loops + NEG_INF mask
    s_all = np.concatenate([src0[keep], np.arange(N, dtype=np.int64)])
    d_all = np.concatenate([dst0[keep], np.arange(N, dtype=np.int64)])
    order = np.argsort(d_all, kind="stable")
    s_all, d_all = s_all[order], d_all[order]

    # per dst-tile slot lists, each padded to a multiple of 128
    chunks = []
    for dt in range(NDT):
        lo, hi = dt * DTW, (dt + 1) * DTW
        m = (d_all >= lo) & (d_all < hi)
        ss, dd = s_all[m], d_all[m]
        cnt = len(ss)
        ntile = max(1, math.ceil(cnt / 128))
        pad = ntile * 128 - cnt
        ss = np.concatenate([ss, np.full(pad, 1000, np.int64)])   # pad -> row 1000
        dd = np.concatenate([dd, np.full(pad, lo, np.int64)])
        real = np.concatenate([np.ones(cnt, bool), np.zeros(pad, bool)])
        # one-hot [p, t, dlocal] (slot j = t*128 + p)
        oh = np.zeros((128, ntile, DTW), np.float32)
        for j in range(ntile * 128):
            if real[j]:
                oh[j % 128, j // 128, dd[j] - lo] = 1.0
        chunks.append(dict(ntile=ntile, src=ss, oh=oh.astype(ml_dtypes.bfloat16),
                           ohT=np.ascontiguousarray(
                               oh.transpose(2, 1, 0)).astype(ml_dtypes.bfloat16)))

    maxt = max(c["ntile"] for c in chunks)
    T = sum(c["ntile"] for c in chunks)
    # index tensor: per chunk, slots wrapped [16, slots/16], replicated to 128
    ihw = np.zeros((128, T * 8), np.int16)   # 128 slots = 8 idx columns
    oh_all = np.zeros((128, T, DTW), ml_dtypes.bfloat16)
    ohT_all = np.zeros((128, T, 128), ml_dtypes.bfloat16)
    t0 = 0
    for c in chunks:
        nt_, ss = c["ntile"], c["src"]
        ni = nt_ * 128
        a = np.zeros((16, ni // 16), np.int16)
        a[np.arange(ni) % 16, np.arange(ni) // 16] = ss.astype(np.int16)
        ihw[:, t0 * 8:(t0 + nt_) * 8] = np.tile(a, (8, 1))
        oh_all[:, t0:t0 + nt_, :] = c["oh"]
        ohT_all[:DTW, t0:t0 + nt_, :] = c["ohT"].reshape(DTW, nt_, 128)
        t0 += nt_
    return {
        "T": T, "maxt": maxt, "ntiles": [c["ntile"] for c in chunks],
        "oh": np.ascontiguousarray(oh_all.reshape(128, T * DTW)),
        "ohT": np.ascontiguousarray(ohT_all.reshape(128, T * 128)),
        "ih": ihw,
    }


def _prep_weights(W, att_src, att_dst):
    W = np.asarray(W, np.float32)
    Ws = np.zeros((F, H), np.float32)
    Wd = np.zeros((F, H), np.float32)
    for h in range(H):
        Ws[:, h] = W[:, h * C:(h + 1) * C] @ np.asarray(att_src, np.float32)[h]
        Wd[:, h] = W[:, h * C:(h + 1) * C] @ np.asarray(att_dst, np.float32)[h]
    # c-major head interleave: device col c*4+h = W col h*64+c
    Wc = np.empty_like(W)
    for h in range(H):
        Wc[:, np.arange(C) * H + h] = W[:, h * C:(h + 1) * C]
    # fold the 1/H head-mean into h (att columns stay unscaled -> z exact)
    waug = np.concatenate([0.25 * Wc, Ws, Wd], axis=1)      # [64, 264]
    return waug.astype(ml_dtypes.bfloat16)


def _make_in_maps(x, W, att_src, att_dst, bias, ed):
    waug = _prep_weights(W, att_src, att_dst)
    bias_slab = np.tile(np.asarray(bias, np.float32)[None, :],
                        (128, RC)).reshape(128, RC * F)
    xr = np.ascontiguousarray(np.asarray(x, np.float32)).reshape(R, N, F)
    in_maps = []
    for cidx in range(NCORES):
        xc = xr[cidx * RC:(cidx + 1) * RC]
        xT = np.ascontiguousarray(xc.transpose(2, 0, 1).reshape(F, RC * N)
                                  ).astype(ml_dtypes.bfloat16)
        in_maps.append({
            "xT": xT, "w_aug": waug, "oh": ed["oh"], "ohT": ed["ohT"],
            "ih": ed["ih"], "bias_slab": bias_slab,
        })
    return in_maps


# --------------------------------------------------------------------------
# device program
# --------------------------------------------------------------------------
def _build_program(ed):
    import concourse.bass as bass
    import concourse.mybir as mybir
    import concourse.tile as tile
    from concourse import bacc

    T, maxt = ed["T"], ed["maxt"]
    ntiles = ed["ntiles"]
    f32 = mybir.dt.float32
    bf16 = mybir.dt.bfloat16
    i16 = mybir.dt.int16
    Alu = mybir.AluOpType
    Act = mybir.ActivationFunctionType

    nc = bacc.Bacc("TRN2", target_bir_lowering=False, debug=False,
                   enable_asserts=False, num_devices=NCORES)

    xT_d = nc.dram_tensor("xT", [F, RC * N], bf16, kind="ExternalInput").ap()
    waug_d = nc.dram_tensor("w_aug", [F, 264], bf16, kind="ExternalInput").ap()
    oh_d = nc.dram_tensor("oh", [128, T * DTW], bf16, kind="ExternalInput").ap()
    ohT_d = nc.dram_tensor("ohT", [128, T * 128], bf16, kind="ExternalInput").ap()
    ih_d = nc.dram_tensor("ih", [128, T * 8], i16, kind="ExternalInput").ap()
    bias_d = nc.dram_tensor("bias_slab", [128, RC * F], f32, kind="ExternalInput").ap()
    out_d = nc.dram_tensor("out", [RC, N, F], f32, kind="ExternalOutput").ap()

    # chunk tile offsets
    t0s = []
    acc = 0
    for nt_ in ntiles:
        t0s.append(acc)
        acc += nt_

    # PSUM layout inside the per-chunk [128, 2048] 4-bank tile:
    DEN0 = 3 * 512            # den [125, 24] at cols 1536:1560 (bank 3)
    EPS0 = DEN0 + AC          # eps [128, maxt, 24] at cols 1560:1560+24*maxt

    with tile.TileContext(nc) as tc:
        with (
            tc.tile_pool(name="const", bufs=1) as constp,
            tc.tile_pool(name="dram", bufs=1, space="DRAM") as dramp,
            # Edge-phase SBUF pools opened BEFORE phase A so the deferred
            # gather writes (prepare_only) never alias phase-A tiles -- Tile
            # does not create WAR edges for deferred prep writes across pool
            # reuse.
            tc.tile_pool(name="hgp", bufs=3) as hgp,
            tc.tile_pool(name="ohp", bufs=2) as ohp,
            tc.tile_pool(name="ohTp", bufs=2) as ohTp,
            tc.tile_pool(name="edge", bufs=2) as edgep,
            tc.tile_pool(name="fin", bufs=2) as finp,
        ):
            h_hbm = dramp.tile([N + 1, ROWW], bf16)

            # projection inputs first (the critical path of phase A),
            # then the gather index tensor (descgen), then the rest.
            waug = constp.tile([F, 264], bf16)
            nc.sync.dma_start(waug[:], waug_d)
            ih = constp.tile([128, T * 8], i16)
            nc.sync.dma_start(ih[:], ih_d)
            bias_sl = constp.tile([128, RC, F], f32)
            nc.sync.dma_start(bias_sl[:], bias_d.rearrange("p (r f) -> p r f", f=F))

            # a_src / a_dst node-space scalars, r-contiguous: [125, NDT, 24]
            as_sb = constp.tile([DTW, NDT, AC], bf16)
            ad_sb = constp.tile([DTW, NDT, AC], bf16)

            # pad row 1000: h-part zeros, as-part -1000 => p == 0 for pad slots
            padrow = constp.tile([1, ROWW], bf16)
            nc.vector.memset(padrow[:], 0.0)
            nc.vector.memset(padrow[:, RC * HC:RC * HC + AC], -1000.0)
            nc.sync.dma_start(h_hbm[N:N + 1, :], padrow[:])

            gsems = [nc.alloc_semaphore(f"gsem{k}") for k in range(NDT)]

            # ---- phase A: projection; fills h_hbm + as_sb/ad_sb ----
            with (
                tc.tile_pool(name="stage", bufs=2) as stagep,
                tc.tile_pool(name="ppsum", bufs=6, space="PSUM") as ppsum,
            ):
                for r in range(RC):
                    xt = stagep.tile([F, N], bf16, tag="xt")
                    nc.sync.dma_start(xt[:], xT_d[:, r * N:(r + 1) * N])
                    hslab = stagep.tile([DTW, NDT, HC], bf16, tag="hslab")
                    for nt in range(NDT):
                        n0 = nt * DTW
                        ps = ppsum.tile([DTW, 264], f32, tag="scratch")
                        nc.tensor.matmul(out=ps[:], lhsT=xt[:, n0:n0 + DTW],
                                         rhs=waug[:], start=True, stop=True)
                        # alternate ACT/DVE so neither serializes phase A
                        if nt % 2 == 0:
                            nc.scalar.copy(out=hslab[:, nt, :], in_=ps[:, 0:HC])
                            nc.vector.tensor_copy(
                                out=as_sb[:, nt, 4 * r:4 * r + 4],
                                in_=ps[:, HC:HC + 4])
                            nc.vector.tensor_copy(
                                out=ad_sb[:, nt, 4 * r:4 * r + 4],
                                in_=ps[:, HC + 4:HC + 8])
                        else:
                            nc.vector.tensor_copy(out=hslab[:, nt, :],
                                                  in_=ps[:, 0:HC])
                            nc.scalar.copy(out=as_sb[:, nt, 4 * r:4 * r + 4],
                                           in_=ps[:, HC:HC + 4])
                            nc.scalar.copy(out=ad_sb[:, nt, 4 * r:4 * r + 4],
                                           in_=ps[:, HC + 4:HC + 8])
                    nc.sync.dma_start(
                        h_hbm[0:N, r * HC:(r + 1) * HC].rearrange(
                            "(a d) e -> d a e", d=DTW), hslab[:])
                # a_src columns of every row: one DMA, 48B contiguous per row
                nc.sync.dma_start(
                    h_hbm[0:N, RC * HC:RC * HC + AC].rearrange(
                        "(a d) e -> d a e", d=DTW), as_sb[:])

            # ---- edge phase ----
            with tc.tile_pool(name="npsum", bufs=2, space="PSUM") as npsum:
                hgs = {}

                def emit_prep(k):
                    nt_ = ntiles[k]
                    ni = nt_ * 128
                    assert ni <= 2032, "gather exceeds SWDGE FIFO"
                    hg = hgp.tile([128, maxt, ROWW], bf16, tag="hg")
                    hgs[k] = hg
                    nc.gpsimd.dma_gather(
                        out_ap=hg[:, 0:nt_, :], in_ap=h_hbm[:],
                        idxs_ap=ih[:, t0s[k] * 8:(t0s[k] + nt_) * 8],
                        num_idxs=ni, num_idxs_reg=ni, elem_size=ROWW,
                        single_packet=False, prepare_only=True, sem=gsems[k])

                emit_prep(0)

                for dt in range(NDT):
                    nt_ = ntiles[dt]
                    t0 = t0s[dt]
                    hg = hgs.pop(dt)
                    nc.gpsimd.trigger_dma(count=None)
                    if dt + 1 < NDT:
                        emit_prep(dt + 1)

                    ohc = ohp.tile([128, maxt, DTW], bf16, tag="oh")
                    nc.sync.dma_start(
                        ohc[:, 0:nt_, :],
                        oh_d[:, t0 * DTW:(t0 + nt_) * DTW].rearrange(
                            "p (t d) -> p t d", d=DTW))
                    ohTc = ohTp.tile([128, maxt, 128], bf16, tag="ohT")
                    nc.sync.dma_start(
                        ohTc[:, 0:nt_, :],
                        ohT_d[:, t0 * 128:(t0 + nt_) * 128].rearrange(
                            "p (t e) -> p t e", e=128))

                    # single 4-bank PSUM tile: num | den | eps
                    nps = npsum.tile([128, 2048], f32, tag="num")

                    # ad expand via transposed one-hot (PE)
                    for t in range(nt_):
                        nc.tensor.matmul(
                            out=nps[:, EPS0 + AC * t:EPS0 + AC * (t + 1)],
                            lhsT=ohTc[0:DTW, t, :],
                            rhs=ad_sb[:, dt, :],
                            start=True, stop=True)

                    # explicit data-readiness gates for the deferred gather
                    nc.vector.wait_ge(gsems[dt], 16)
                    nc.scalar.wait_ge(gsems[dt], 16)
                    nc.tensor.wait_ge(gsems[dt], 16)

                    # z = as + ad ; leaky relu ; exp
                    z = edgep.tile([128, maxt, AC], f32, tag="z")
                    epsv = nps[:, EPS0:EPS0 + AC * nt_].rearrange(
                        "p (t a) -> p t a", a=AC)
                    nc.vector.tensor_tensor(
                        out=z[:, 0:nt_, :],
                        in0=hg[:, 0:nt_, RC * HC:RC * HC + AC],
                        in1=epsv, op=Alu.add)
                    nc.vector.scalar_tensor_tensor(
                        out=z[:, 0:nt_, :], in0=z[:, 0:nt_, :], scalar=NEG_SLOPE,
                        in1=z[:, 0:nt_, :], op0=Alu.mult, op1=Alu.max)
                    p_bf = edgep.tile([128, maxt, AC], bf16, tag="p")
                    nc.scalar.activation(out=p_bf[:, 0:nt_, :], in_=z[:, 0:nt_, :],
                                         func=Act.Exp)

                    # den = segsum(p) -- early, overlaps the num matmuls
                    for t in range(nt_):
                        nc.tensor.matmul(out=nps[0:DTW, DEN0:DEN0 + AC],
                                         lhsT=ohc[:, t, :],
                                         rhs=p_bf[:, t, :],
                                         start=(t == 0), stop=(t == nt_ - 1))
                    drb = finp.tile([DTW, AC], f32, tag="drb")
                    nc.vector.reciprocal(out=drb[:], in_=nps[0:DTW, DEN0:DEN0 + AC])

                    # msg = hg * p (in-place, bf16 DVE)
                    for r in range(RC):
                        hgr = hg[:, 0:nt_, r * HC:(r + 1) * HC].rearrange(
                            "p t (c h) -> p t c h", h=H)
                        pb = p_bf[:, 0:nt_, 4 * r:4 * r + 4].rearrange(
                            "p t (o h) -> p t o h", o=1).to_broadcast(
                            [128, nt_, C, H])
                        nc.vector.tensor_tensor(out=hgr, in0=hgr, in1=pb,
                                                op=Alu.mult)

                    # num = segsum(msg): 3x512-col accumulating matmuls per tile
                    for t in range(nt_):
                        for cb in range(3):
                            nc.tensor.matmul(
                                out=nps[0:DTW, cb * 512:(cb + 1) * 512],
                                lhsT=ohc[:, t, :],
                                rhs=hg[:, t, cb * 512:(cb + 1) * 512],
                                start=(t == 0), stop=(t == nt_ - 1))

                    # finalize: numn = num * (1/den), h-major bf16 so the
                    # head-sums run in the DVE fast mode
                    numn = finp.tile([DTW, RC, H, C], bf16, tag="numn")
                    drbb = drb[:].rearrange("d (r o h) -> d r o h", h=H, o=1
                                            ).to_broadcast([DTW, RC, C, H])
                    nc.vector.tensor_tensor(
                        out=numn[:].rearrange("d r h c -> d r c h"),
                        in0=nps[0:DTW, 0:RC * HC].rearrange(
                            "d (r c h) -> d r c h", h=H, c=C),
                        in1=drbb, op=Alu.mult)
                    t12 = finp.tile([DTW, RC, C], bf16, tag="t12")
                    t34 = finp.tile([DTW, RC, C], bf16, tag="t34")
                    ob = finp.tile([DTW, RC, C], f32, tag="ob")
                    nc.vector.tensor_tensor(out=t12[:], in0=numn[:, :, 0, :],
                                            in1=numn[:, :, 1, :], op=Alu.add)
                    nc.vector.tensor_tensor(out=t34[:], in0=numn[:, :, 2, :],
                                            in1=numn[:, :, 3, :], op=Alu.add)
                    nc.vector.tensor_tensor(out=ob[:], in0=t12[:], in1=t34[:],
                                            op=Alu.add)
                    nc.vector.tensor_tensor(out=ob[:], in0=ob[:],
                                            in1=bias_sl[0:DTW, :, :], op=Alu.add)
                    nc.sync.dma_start(
                        out_d[:, dt * DTW:(dt + 1) * DTW, :].rearrange(
                            "r d f -> d r f"), ob[:])

    nc.compile()
    return nc


# --------------------------------------------------------------------------
# public entry point
# --------------------------------------------------------------------------
def kernel(x, edge_index, W, att_src, att_dst, bias):
    key = hash(np.asarray(edge_index).tobytes())
    if key not in _CACHE:
        ed = _prep_edges(edge_index)
        _CACHE[key] = (_build_program(ed), ed)
    nc, ed = _CACHE[key]

    in_maps = _make_in_maps(x, W, att_src, att_dst, bias, ed)
    from concourse import bass_utils
    res = bass_utils.run_bass_kernel_spmd(nc, in_maps, core_ids=list(range(NCORES)))
    outs = [res.results[c]["out"] for c in range(NCORES)]
    out = np.concatenate(outs, axis=0).reshape(B, S, N, F).astype(np.float32)
    return out


# revision 16
# speedup vs baseline: 1.2839x; 1.0867x over previous
"""GAT layer kernel for Trainium2, 8 NeuronCores, data-parallel over R=b*s.

Self-contained: takes full inputs, returns full output.

v3.1 design (per core, RC=6 replicas):
  - Projection on PE in bf16 (x, W host-cast); the 1/4 head-mean factor is
    folded into the projection weights.  6-deep PSUM pipeline keeps the PE
    at full clock; PSUM->SBUF copies alternate ACT/DVE.
  - h (bf16) plus per-node a_src scalars are written into ONE HBM row per
    node: row n = [h r0..r5 (6*256 bf16, c-major) | a_src 24 bf16 | pad]
    (3328 B).  a_dst scalars stay in SBUF (r-contiguous).
  - Edge phase chunked by dst-tile (125 dsts, dst-sorted slots padded to
    128-slot tiles).  Per chunk ONE dma_gather on a SINGLE SWDGE queue
    (sequential completion = pipelined compute).  Descriptor generation is
    front-loaded with prepare_only (it only needs the index tensor), each
    transfer fired by trigger_dma once h is in HBM.  Data-readiness for
    consumers is enforced by explicit per-engine wait_ge on the gather
    semaphore (Tile's automatic consumer waits fire at descgen completion,
    not DMA completion, for deferred preps).
  - z = a_src[src] (gathered) + a_dst[dst] (PE expand via transposed one-hot)
    p = exp(leaky_relu(z)); den = segment_sum(p) (PE one-hot) right after exp
    so the reciprocal overlaps the num matmuls.
  - msg = hg * p (DVE bf16, heads broadcast over c-major layout)
    num = segment_sum(msg): 3x512-col accumulating matmuls per tile into a
    single 4-bank PSUM tile that also holds den and the eps scratch.
  - finalize: one DVE pass over PSUM (num * (1/den) -> bf16 h-major), 4x-mode
    head-sums, + bias, DMA out.
"""

import math
import numpy as np
import ml_dtypes

B, S, N, F = 4, 12, 1000, 64
H, C = 4, 64
HC = H * C            # 256
R = B * S             # 48
NCORES = 8
RC = R // NCORES      # 6 replicas per core
NEG_SLOPE = 0.2
DTW = 125             # dst-tile width (8 tiles cover N=1000)
NDT = N // DTW        # 8
AC = RC * H           # 24 active scalar columns
ROWW = 1664           # h_hbm row width in bf16 (6*256 h + 24 as + pad) = 3328B

_CACHE = {}


# --------------------------------------------------------------------------
# host-side index preprocessing
# --------------------------------------------------------------------------
def _prep_edges(edge_index):
    src0 = np.asarray(edge_index[0], dtype=np.int64)
    dst0 = np.asarray(edge_index[1], dtype=np.int64)
    keep = src0 != dst0                      # PyG remove_self_loops + NEG_INF mask
    s_all = np.concatenate([src0[keep], np.arange(N, dtype=np.int64)])
    d_all = np.concatenate([dst0[keep], np.arange(N, dtype=np.int64)])
    order = np.argsort(d_all, kind="stable")
    s_all, d_all = s_all[order], d_all[order]

    # per dst-tile slot lists, each padded to a multiple of 128
    chunks = []
    for dt in range(NDT):
        lo, hi = dt * DTW, (dt + 1) * DTW
        m = (d_all >= lo) & (d_all < hi)
        ss, dd = s_all[m], d_all[m]
        cnt = len(ss)
        ntile = max(1, math.ceil(cnt / 128))
        pad = ntile * 128 - cnt
        ss = np.concatenate([ss, np.full(pad, 1000, np.int64)])   # pad -> row 1000
        dd = np.concatenate([dd, np.full(pad, lo, np.int64)])
        real = np.concatenate([np.ones(cnt, bool), np.zeros(pad, bool)])
        # one-hot [p, t, dlocal] (slot j = t*128 + p)
        oh = np.zeros((128, ntile, DTW), np.float32)
        for j in range(ntile * 128):
            if real[j]:
                oh[j % 128, j // 128, dd[j] - lo] = 1.0
        chunks.append(dict(ntile=ntile, src=ss, oh=oh.astype(ml_dtypes.bfloat16),
                           ohT=np.ascontiguousarray(
                               oh.transpose(2, 1, 0)).astype(ml_dtypes.bfloat16)))

    maxt = max(c["ntile"] for c in chunks)
    T = sum(c["ntile"] for c in chunks)
    # index tensor: per chunk, slots wrapped [16, slots/16], replicated to 128
    ihw = np.zeros((128, T * 8), np.int16)   # 128 slots = 8 idx columns
    oh_all = np.zeros((128, T, DTW), ml_dtypes.bfloat16)
    ohT_all = np.zeros((128, T, 128), ml_dtypes.bfloat16)
    t0 = 0
    for c in chunks:
        nt_, ss = c["ntile"], c["src"]
        ni = nt_ * 128
        a = np.zeros((16, ni // 16), np.int16)
        a[np.arange(ni) % 16, np.arange(ni) // 16] = ss.astype(np.int16)
        ihw[:, t0 * 8:(t0 + nt_) * 8] = np.tile(a, (8, 1))
        oh_all[:, t0:t0 + nt_, :] = c["oh"]
        ohT_all[:DTW, t0:t0 + nt_, :] = c["ohT"].reshape(DTW, nt_, 128)
        t0 += nt_
    return {
        "T": T, "maxt": maxt, "ntiles": [c["ntile"] for c in chunks],
        "oh": np.ascontiguousarray(oh_all.reshape(128, T * DTW)),
        "ohT": np.ascontiguousarray(ohT_all.reshape(128, T * 128)),
        "ih": ihw,
    }


def _prep_weights(W, att_src, att_dst):
    W = np.asarray(W, np.float32)
    Ws = np.zeros((F, H), np.float32)
    Wd = np.zeros((F, H), np.float32)
    for h in range(H):
        Ws[:, h] = W[:, h * C:(h + 1) * C] @ np.asarray(att_src, np.float32)[h]
        Wd[:, h] = W[:, h * C:(h + 1) * C] @ np.asarray(att_dst, np.float32)[h]
    # c-major head interleave: device col c*4+h = W col h*64+c
    Wc = np.empty_like(W)
    for h in range(H):
        Wc[:, np.arange(C) * H + h] = W[:, h * C:(h + 1) * C]
    # fold the 1/H head-mean into h (att columns stay unscaled -> z exact)
    waug = np.concatenate([0.25 * Wc, Ws, Wd], axis=1)      # [64, 264]
    return waug.astype(ml_dtypes.bfloat16)


def _make_in_maps(x, W, att_src, att_dst, bias, ed):
    waug = _prep_weights(W, att_src, att_dst)
    bias_slab = np.tile(np.asarray(bias, np.float32)[None, :],
                        (128, RC)).reshape(128, RC * F)
    xr = np.ascontiguousarray(np.asarray(x, np.float32)).reshape(R, N, F)
    in_maps = []
    for cidx in range(NCORES):
        xc = xr[cidx * RC:(cidx + 1) * RC]
        xT = np.ascontiguousarray(xc.transpose(2, 0, 1).reshape(F, RC * N)
                                  ).astype(ml_dtypes.bfloat16)
        in_maps.append({
            "xT": xT, "w_aug": waug, "oh": ed["oh"], "ohT": ed["ohT"],
            "ih": ed["ih"], "bias_slab": bias_slab,
        })
    return in_maps


# --------------------------------------------------------------------------
# device program
# --------------------------------------------------------------------------
def _build_program(ed):
    import concourse.bass as bass
    import concourse.mybir as mybir
    import concourse.tile as tile
    from concourse import bacc

    T, maxt = ed["T"], ed["maxt"]
    ntiles = ed["ntiles"]
    f32 = mybir.dt.float32
    bf16 = mybir.dt.bfloat16
    i16 = mybir.dt.int16
    Alu = mybir.AluOpType
    Act = mybir.ActivationFunctionType

    nc = bacc.Bacc("TRN2", target_bir_lowering=False, debug=False,
                   enable_asserts=False, num_devices=NCORES)

    xT_d = nc.dram_tensor("xT", [F, RC * N], bf16, kind="ExternalInput").ap()
    waug_d = nc.dram_tensor("w_aug", [F, 264], bf16, kind="ExternalInput").ap()
    oh_d = nc.dram_tensor("oh", [128, T * DTW], bf16, kind="ExternalInput").ap()
    ohT_d = nc.dram_tensor("ohT", [128, T * 128], bf16, kind="ExternalInput").ap()
    ih_d = nc.dram_tensor("ih", [128, T * 8], i16, kind="ExternalInput").ap()
    bias_d = nc.dram_tensor("bias_slab", [128, RC * F], f32, kind="ExternalInput").ap()
    out_d = nc.dram_tensor("out", [RC, N, F], f32, kind="ExternalOutput").ap()

    # chunk tile offsets
    t0s = []
    acc = 0
    for nt_ in ntiles:
        t0s.append(acc)
        acc += nt_

    # PSUM layout inside the per-chunk [128, 2048] 4-bank tile:
    DEN0 = 3 * 512            # den [125, 24] at cols 1536:1560 (bank 3)
    EPS0 = DEN0 + AC          # eps [128, maxt, 24] at cols 1560:1560+24*maxt

    with tile.TileContext(nc) as tc:
        with (
            tc.tile_pool(name="const", bufs=1) as constp,
            tc.tile_pool(name="dram", bufs=1, space="DRAM") as dramp,
            # Edge-phase SBUF pools opened BEFORE phase A so the deferred
            # gather writes (prepare_only) never alias phase-A tiles -- Tile
            # does not create WAR edges for deferred prep writes across pool
            # reuse.
            tc.tile_pool(name="hgp", bufs=3) as hgp,
            tc.tile_pool(name="ohp", bufs=2) as ohp,
            tc.tile_pool(name="ohTp", bufs=2) as ohTp,
            tc.tile_pool(name="edge", bufs=2) as edgep,
            tc.tile_pool(name="fin", bufs=2) as finp,
        ):
            h_hbm = dramp.tile([N + 1, ROWW], bf16)

            # projection inputs first (the critical path of phase A),
            # then the gather index tensor (descgen), then the rest.
            waug = constp.tile([F, 264], bf16)
            nc.sync.dma_start(waug[:], waug_d)
            ih = constp.tile([128, T * 8], i16)
            nc.sync.dma_start(ih[:], ih_d)
            bias_sl = constp.tile([128, RC, F], f32)
            nc.sync.dma_start(bias_sl[:], bias_d.rearrange("p (r f) -> p r f", f=F))

            # a_src node-space scalars, r-contiguous: [125, NDT, 24]
            as_sb = constp.tile([DTW, NDT, AC], bf16)
            # full projection slab: h + a_src + a_dst per (node, replica)
            hslab = constp.tile([DTW, RC, NDT, 264], bf16)

            # pad row 1000: h-part zeros, as-part -1000 => p == 0 for pad slots
            padrow = constp.tile([1, ROWW], bf16)
            nc.vector.memset(padrow[:], 0.0)
            nc.vector.memset(padrow[:, RC * HC:RC * HC + AC], -1000.0)
            nc.sync.dma_start(h_hbm[N:N + 1, :], padrow[:])

            gsems = [nc.alloc_semaphore(f"gsem{k}") for k in range(NDT)]

            # ---- phase A: projection; fills h_hbm + as_sb/ad_sb ----
            with (
                tc.tile_pool(name="stage", bufs=2) as stagep,
                tc.tile_pool(name="ppsum", bufs=6, space="PSUM") as ppsum,
            ):
                for r in range(RC):
                    xt = stagep.tile([F, N], bf16, tag="xt")
                    nc.sync.dma_start(xt[:], xT_d[:, r * N:(r + 1) * N])
                    for nt in range(NDT):
                        n0 = nt * DTW
                        ps = ppsum.tile([DTW, 264], f32, tag="scratch")
                        nc.tensor.matmul(out=ps[:], lhsT=xt[:, n0:n0 + DTW],
                                         rhs=waug[:], start=True, stop=True)
                        # one full-width copy; alternate ACT/DVE
                        if nt % 2 == 0:
                            nc.scalar.copy(out=hslab[:, r, nt, :], in_=ps[:])
                        else:
                            nc.vector.tensor_copy(out=hslab[:, r, nt, :],
                                                  in_=ps[:])
                    nc.sync.dma_start(
                        h_hbm[0:N, r * HC:(r + 1) * HC].rearrange(
                            "(a d) e -> d a e", d=DTW), hslab[:, r, :, 0:HC])
                # r-contiguous a_src: one strided DVE gather-copy, one DMA
                nc.vector.tensor_copy(
                    out=as_sb[:].rearrange("d a (r e) -> d a r e", e=4),
                    in_=hslab[:, :, :, HC:HC + 4].rearrange(
                        "d r a e -> d a r e"))
                nc.sync.dma_start(
                    h_hbm[0:N, RC * HC:RC * HC + AC].rearrange(
                        "(a d) e -> d a e", d=DTW), as_sb[:])

            # ---- edge phase ----
            with tc.tile_pool(name="npsum", bufs=2, space="PSUM") as npsum:
                for dt in range(NDT):
                    nt_ = ntiles[dt]
                    t0 = t0s[dt]
                    ni = nt_ * 128
                    assert ni <= 2032, "gather exceeds SWDGE FIFO"
                    hg = hgp.tile([128, maxt, ROWW], bf16, tag="hg")
                    nc.gpsimd.dma_gather(
                        out_ap=hg[:, 0:nt_, :], in_ap=h_hbm[:],
                        idxs_ap=ih[:, t0 * 8:(t0 + nt_) * 8],
                        num_idxs=ni, num_idxs_reg=ni, elem_size=ROWW,
                        single_packet=False).then_inc(gsems[dt], 16)

                    ohc = ohp.tile([128, maxt, DTW], bf16, tag="oh")
                    nc.sync.dma_start(
                        ohc[:, 0:nt_, :],
                        oh_d[:, t0 * DTW:(t0 + nt_) * DTW].rearrange(
                            "p (t d) -> p t d", d=DTW))
                    ohTc = ohTp.tile([128, maxt, 128], bf16, tag="ohT")
                    nc.sync.dma_start(
                        ohTc[:, 0:nt_, :],
                        ohT_d[:, t0 * 128:(t0 + nt_) * 128].rearrange(
                            "p (t e) -> p t e", e=128))

                    # single 4-bank PSUM tile: num | den | eps
                    nps = npsum.tile([128, 2048], f32, tag="num")

                    # ad expand via transposed one-hot (PE); rhs reads
                    # the a_dst columns of the projection slab (strided)
                    for t in range(nt_):
                        nc.tensor.matmul(
                            out=nps[:, EPS0 + AC * t:EPS0 + AC * (t + 1)],
                            lhsT=ohTc[0:DTW, t, :],
                            rhs=hslab[:, :, dt, HC + 4:HC + 8],
                            start=True, stop=True)

                    # stage eps out of PSUM (ACT is idle; DVE reads of
                    # PSUM are slow)
                    eps_sb = edgep.tile([128, maxt, AC], f32, tag="eps")
                    nc.scalar.copy(
                        out=eps_sb[:, 0:nt_, :],
                        in_=nps[:, EPS0:EPS0 + AC * nt_].rearrange(
                            "p (t a) -> p t a", a=AC))

                    # explicit data-readiness gates for the deferred gather
                    nc.vector.wait_ge(gsems[dt], 16)
                    nc.scalar.wait_ge(gsems[dt], 16)
                    nc.tensor.wait_ge(gsems[dt], 16)

                    # z = as + ad ; leaky relu ; exp
                    z = edgep.tile([128, maxt, AC], f32, tag="z")
                    nc.vector.tensor_tensor(
                        out=z[:, 0:nt_, :],
                        in0=hg[:, 0:nt_, RC * HC:RC * HC + AC],
                        in1=eps_sb[:, 0:nt_, :], op=Alu.add)
                    nc.vector.scalar_tensor_tensor(
                        out=z[:, 0:nt_, :], in0=z[:, 0:nt_, :], scalar=NEG_SLOPE,
                        in1=z[:, 0:nt_, :], op0=Alu.mult, op1=Alu.max)
                    p_bf = edgep.tile([128, maxt, AC], bf16, tag="p")
                    nc.scalar.activation(out=p_bf[:, 0:nt_, :], in_=z[:, 0:nt_, :],
                                         func=Act.Exp)

                    # den = segsum(p) -- early, overlaps the num matmuls
                    for t in range(nt_):
                        nc.tensor.matmul(out=nps[0:DTW, DEN0:DEN0 + AC],
                                         lhsT=ohc[:, t, :],
                                         rhs=p_bf[:, t, :],
                                         start=(t == 0), stop=(t == nt_ - 1))
                    drb = finp.tile([DTW, AC], f32, tag="drb")
                    nc.vector.reciprocal(out=drb[:], in_=nps[0:DTW, DEN0:DEN0 + AC])

                    # msg = hg * p (in-place, bf16 DVE)
                    for r in range(RC):
                        hgr = hg[:, 0:nt_, r * HC:(r + 1) * HC].rearrange(
                            "p t (c h) -> p t c h", h=H)
                        pb = p_bf[:, 0:nt_, 4 * r:4 * r + 4].rearrange(
                            "p t (o h) -> p t o h", o=1).to_broadcast(
                            [128, nt_, C, H])
                        nc.vector.tensor_tensor(out=hgr, in0=hgr, in1=pb,
                                                op=Alu.mult)

                    # num = segsum(msg): 3x512-col accumulating matmuls per tile
                    for t in range(nt_):
                        for cb in range(3):
                            nc.tensor.matmul(
                                out=nps[0:DTW, cb * 512:(cb + 1) * 512],
                                lhsT=ohc[:, t, :],
                                rhs=hg[:, t, cb * 512:(cb + 1) * 512],
                                start=(t == 0), stop=(t == nt_ - 1))

                    # finalize: numn = num * (1/den), h-major bf16 so the
                    # head-sums run in the DVE fast mode
                    numn = finp.tile([DTW, RC, C, H], bf16, tag="numn")
                    drbb = drb[:].rearrange("d (r o h) -> d r o h", h=H, o=1
                                            ).to_broadcast([DTW, RC, C, H])
                    nc.vector.tensor_tensor(
                        out=numn[:],
                        in0=nps[0:DTW, 0:RC * HC].rearrange(
                            "d (r c h) -> d r c h", h=H, c=C),
                        in1=drbb, op=Alu.mult)
                    t12 = finp.tile([DTW, RC, C], bf16, tag="t12")
                    t34 = finp.tile([DTW, RC, C], bf16, tag="t34")
                    ob = finp.tile([DTW, RC, C], f32, tag="ob")
                    nc.vector.tensor_tensor(out=t12[:], in0=numn[:, :, :, 0],
                                            in1=numn[:, :, :, 1], op=Alu.add)
                    nc.vector.tensor_tensor(out=t34[:], in0=numn[:, :, :, 2],
                                            in1=numn[:, :, :, 3], op=Alu.add)
                    nc.vector.tensor_tensor(out=ob[:], in0=t12[:], in1=t34[:],
                                            op=Alu.add)
                    nc.vector.tensor_tensor(out=ob[:], in0=ob[:],
                                            in1=bias_sl[0:DTW, :, :], op=Alu.add)
                    nc.sync.dma_start(
                        out_d[:, dt * DTW:(dt + 1) * DTW, :].rearrange(
                            "r d f -> d r f"), ob[:])

    nc.compile()
    return nc


# --------------------------------------------------------------------------
# public entry point
# --------------------------------------------------------------------------
def kernel(x, edge_index, W, att_src, att_dst, bias):
    key = hash(np.asarray(edge_index).tobytes())
    if key not in _CACHE:
        ed = _prep_edges(edge_index)
        _CACHE[key] = (_build_program(ed), ed)
    nc, ed = _CACHE[key]

    in_maps = _make_in_maps(x, W, att_src, att_dst, bias, ed)
    from concourse import bass_utils
    res = bass_utils.run_bass_kernel_spmd(nc, in_maps, core_ids=list(range(NCORES)))
    outs = [res.results[c]["out"] for c in range(NCORES)]
    out = np.concatenate(outs, axis=0).reshape(B, S, N, F).astype(np.float32)
    return out


# revision 21
# speedup vs baseline: 1.4304x; 1.1141x over previous
"""GAT layer kernel for Trainium2, 8 NeuronCores, data-parallel over R=b*s.

Self-contained: takes full inputs, returns full output.

v3.1 design (per core, RC=6 replicas):
  - Projection on PE in bf16 (x, W host-cast); the 1/4 head-mean factor is
    folded into the projection weights.  6-deep PSUM pipeline keeps the PE
    at full clock; PSUM->SBUF copies alternate ACT/DVE.
  - h (bf16) plus per-node a_src scalars are written into ONE HBM row per
    node: row n = [h r0..r5 (6*256 bf16, c-major) | a_src 24 bf16 | pad]
    (3328 B).  a_dst scalars stay in SBUF (r-contiguous).
  - Edge phase chunked by dst-tile (125 dsts, dst-sorted slots padded to
    128-slot tiles).  Per chunk ONE dma_gather on a SINGLE SWDGE queue
    (sequential completion = pipelined compute).  Descriptor generation is
    front-loaded with prepare_only (it only needs the index tensor), each
    transfer fired by trigger_dma once h is in HBM.  Data-readiness for
    consumers is enforced by explicit per-engine wait_ge on the gather
    semaphore (Tile's automatic consumer waits fire at descgen completion,
    not DMA completion, for deferred preps).
  - z = a_src[src] (gathered) + a_dst[dst] (PE expand via transposed one-hot)
    p = exp(leaky_relu(z)); den = segment_sum(p) (PE one-hot) right after exp
    so the reciprocal overlaps the num matmuls.
  - msg = hg * p (DVE bf16, heads broadcast over c-major layout)
    num = segment_sum(msg): 3x512-col accumulating matmuls per tile into a
    single 4-bank PSUM tile that also holds den and the eps scratch.
  - finalize: one DVE pass over PSUM (num * (1/den) -> bf16 h-major), 4x-mode
    head-sums, + bias, DMA out.
"""

import math
import numpy as np
import ml_dtypes

B, S, N, F = 4, 12, 1000, 64
H, C = 4, 64
HC = H * C            # 256
R = B * S             # 48
NCORES = 8
RC = R // NCORES      # 6 replicas per core
NEG_SLOPE = 0.2
DTW = 125             # dst-tile width (8 tiles cover N=1000)
NDT = N // DTW        # 8
AC = RC * H           # 24 active scalar columns
ROWW = 1664           # h_hbm row width in bf16 (6*256 h + 24 as + pad) = 3328B

_CACHE = {}


# --------------------------------------------------------------------------
# host-side index preprocessing
# --------------------------------------------------------------------------
def _prep_edges(edge_index):
    src0 = np.asarray(edge_index[0], dtype=np.int64)
    dst0 = np.asarray(edge_index[1], dtype=np.int64)
    keep = src0 != dst0                      # PyG remove_self_loops + NEG_INF mask
    s_all = np.concatenate([src0[keep], np.arange(N, dtype=np.int64)])
    d_all = np.concatenate([dst0[keep], np.arange(N, dtype=np.int64)])
    order = np.argsort(d_all, kind="stable")
    s_all, d_all = s_all[order], d_all[order]

    # per dst-tile slot lists, each padded to a multiple of 128
    chunks = []
    for dt in range(NDT):
        lo, hi = dt * DTW, (dt + 1) * DTW
        m = (d_all >= lo) & (d_all < hi)
        ss, dd = s_all[m], d_all[m]
        cnt = len(ss)
        ntile = max(1, math.ceil(cnt / 128))
        pad = ntile * 128 - cnt
        ss = np.concatenate([ss, np.full(pad, 1000, np.int64)])   # pad -> row 1000
        dd = np.concatenate([dd, np.full(pad, lo, np.int64)])
        real = np.concatenate([np.ones(cnt, bool), np.zeros(pad, bool)])
        # one-hot [p, t, dlocal] (slot j = t*128 + p)
        oh = np.zeros((128, ntile, DTW), np.float32)
        for j in range(ntile * 128):
            if real[j]:
                oh[j % 128, j // 128, dd[j] - lo] = 1.0
        chunks.append(dict(ntile=ntile, src=ss, oh=oh.astype(ml_dtypes.bfloat16),
                           ohT=np.ascontiguousarray(
                               oh.transpose(2, 1, 0)).astype(ml_dtypes.bfloat16)))

    maxt = max(c["ntile"] for c in chunks)
    T = sum(c["ntile"] for c in chunks)
    # index tensor: per chunk, slots wrapped [16, slots/16], replicated to 128
    ihw = np.zeros((128, T * 8), np.int16)   # 128 slots = 8 idx columns
    oh_all = np.zeros((128, T, DTW), ml_dtypes.bfloat16)
    ohT_all = np.zeros((128, T, 128), ml_dtypes.bfloat16)
    t0 = 0
    for c in chunks:
        nt_, ss = c["ntile"], c["src"]
        ni = nt_ * 128
        a = np.zeros((16, ni // 16), np.int16)
        a[np.arange(ni) % 16, np.arange(ni) // 16] = ss.astype(np.int16)
        ihw[:, t0 * 8:(t0 + nt_) * 8] = np.tile(a, (8, 1))
        oh_all[:, t0:t0 + nt_, :] = c["oh"]
        ohT_all[:DTW, t0:t0 + nt_, :] = c["ohT"].reshape(DTW, nt_, 128)
        t0 += nt_
    return {
        "T": T, "maxt": maxt, "ntiles": [c["ntile"] for c in chunks],
        "oh": np.ascontiguousarray(oh_all.reshape(128, T * DTW)),
        "ohT": np.ascontiguousarray(ohT_all.reshape(128, T * 128)),
        "ih": ihw,
    }


def _prep_weights(W, att_src, att_dst):
    W = np.asarray(W, np.float32)
    Ws = np.zeros((F, H), np.float32)
    Wd = np.zeros((F, H), np.float32)
    for h in range(H):
        Ws[:, h] = W[:, h * C:(h + 1) * C] @ np.asarray(att_src, np.float32)[h]
        Wd[:, h] = W[:, h * C:(h + 1) * C] @ np.asarray(att_dst, np.float32)[h]
    # c-major head interleave: device col c*4+h = W col h*64+c
    Wc = np.empty_like(W)
    for h in range(H):
        Wc[:, np.arange(C) * H + h] = W[:, h * C:(h + 1) * C]
    # fold the 1/H head-mean into h (att columns stay unscaled -> z exact)
    waug = np.concatenate([0.25 * Wc, Ws, Wd], axis=1)      # [64, 264]
    return waug.astype(ml_dtypes.bfloat16)


def _make_in_maps(x, W, att_src, att_dst, bias, ed):
    waug = _prep_weights(W, att_src, att_dst)
    bias_slab = np.tile(np.asarray(bias, np.float32)[None, :],
                        (128, RC)).reshape(128, RC * F)
    xr = np.ascontiguousarray(np.asarray(x, np.float32)).reshape(R, N, F)
    in_maps = []
    for cidx in range(NCORES):
        xc = xr[cidx * RC:(cidx + 1) * RC]
        xT = np.ascontiguousarray(xc.transpose(2, 0, 1).reshape(F, RC * N)
                                  ).astype(ml_dtypes.bfloat16)
        in_maps.append({
            "xT": xT, "w_aug": waug, "oh": ed["oh"], "ohT": ed["ohT"],
            "ih": ed["ih"], "bias_slab": bias_slab,
        })
    return in_maps


# --------------------------------------------------------------------------
# device program
# --------------------------------------------------------------------------
def _build_program(ed):
    import concourse.bass as bass
    import concourse.mybir as mybir
    import concourse.tile as tile
    from concourse import bacc

    T, maxt = ed["T"], ed["maxt"]
    ntiles = ed["ntiles"]
    f32 = mybir.dt.float32
    bf16 = mybir.dt.bfloat16
    i16 = mybir.dt.int16
    Alu = mybir.AluOpType
    Act = mybir.ActivationFunctionType

    nc = bacc.Bacc("TRN2", target_bir_lowering=False, debug=False,
                   enable_asserts=False, num_devices=NCORES)

    xT_d = nc.dram_tensor("xT", [F, RC * N], bf16, kind="ExternalInput").ap()
    waug_d = nc.dram_tensor("w_aug", [F, 264], bf16, kind="ExternalInput").ap()
    oh_d = nc.dram_tensor("oh", [128, T * DTW], bf16, kind="ExternalInput").ap()
    ohT_d = nc.dram_tensor("ohT", [128, T * 128], bf16, kind="ExternalInput").ap()
    ih_d = nc.dram_tensor("ih", [128, T * 8], i16, kind="ExternalInput").ap()
    bias_d = nc.dram_tensor("bias_slab", [128, RC * F], f32, kind="ExternalInput").ap()
    out_d = nc.dram_tensor("out", [RC, N, F], f32, kind="ExternalOutput").ap()

    # chunk tile offsets
    t0s = []
    acc = 0
    for nt_ in ntiles:
        t0s.append(acc)
        acc += nt_

    # PSUM layout inside the per-chunk [128, 2048] 4-bank tile:
    DEN0 = 3 * 512            # den [125, 24] at cols 1536:1560 (bank 3)
    EPS0 = DEN0 + AC          # eps [128, maxt, 24] at cols 1560:1560+24*maxt

    with tile.TileContext(nc) as tc:
        with (
            tc.tile_pool(name="const", bufs=1) as constp,
            tc.tile_pool(name="dram", bufs=1, space="DRAM") as dramp,
            # Edge-phase SBUF pools opened BEFORE phase A so the deferred
            # gather writes (prepare_only) never alias phase-A tiles -- Tile
            # does not create WAR edges for deferred prep writes across pool
            # reuse.
            tc.tile_pool(name="hgp", bufs=6) as hgp,
            tc.tile_pool(name="ohp", bufs=4) as ohp,
            tc.tile_pool(name="ohTp", bufs=2) as ohTp,
            tc.tile_pool(name="edge", bufs=4) as edgep,
            tc.tile_pool(name="fin", bufs=2) as finp,
        ):
            h_hbm = dramp.tile([N + 1, ROWW], bf16)

            # projection inputs first (the critical path of phase A),
            # then the gather index tensor (descgen), then the rest.
            waug = constp.tile([F, 264], bf16)
            nc.sync.dma_start(waug[:], waug_d)
            # issue side constants from other engines' queues: the Sync queue
            # carries only waug + the xt loads (phase A critical path)
            ih = constp.tile([128, T * 8], i16)
            nc.scalar.dma_start(ih[:], ih_d)
            bias_sl = constp.tile([128, RC, F], f32)
            nc.scalar.dma_start(bias_sl[:], bias_d.rearrange("p (r f) -> p r f", f=F))

            # a_src node-space scalars, r-contiguous: [125, NDT, 24]
            as_sb = constp.tile([DTW, NDT, AC], bf16)
            # full projection slab: h + a_src + a_dst per (node, replica)
            hslab = constp.tile([DTW, RC, NDT, 264], bf16)

            # pad row 1000: h-part zeros, as-part -1000 => p == 0 for pad slots
            padrow = constp.tile([1, ROWW], bf16)
            nc.vector.memset(padrow[:], 0.0)
            nc.vector.memset(padrow[:, RC * HC:RC * HC + AC], -1000.0)
            nc.scalar.dma_start(h_hbm[N:N + 1, :], padrow[:])

            gsems = [nc.alloc_semaphore(f"gsem{k}") for k in range(2 * NDT)]

            # ---- phase A: projection; fills h_hbm + as_sb/ad_sb ----
            with (
                tc.tile_pool(name="stage", bufs=2) as stagep,
                tc.tile_pool(name="ppsum", bufs=6, space="PSUM") as ppsum,
            ):
                for r in range(RC):
                    xt = stagep.tile([F, N], bf16, tag="xt")
                    nc.sync.dma_start(xt[:], xT_d[:, r * N:(r + 1) * N])
                    for nt in range(NDT):
                        n0 = nt * DTW
                        ps = ppsum.tile([DTW, 264], f32, tag="scratch")
                        nc.tensor.matmul(out=ps[:], lhsT=xt[:, n0:n0 + DTW],
                                         rhs=waug[:], start=True, stop=True)
                        # one full-width copy; alternate ACT/DVE
                        if nt % 2 == 0:
                            nc.scalar.copy(out=hslab[:, r, nt, :], in_=ps[:])
                        else:
                            nc.vector.tensor_copy(out=hslab[:, r, nt, :],
                                                  in_=ps[:])
                    nc.gpsimd.dma_start(
                        h_hbm[0:N, r * HC:(r + 1) * HC].rearrange(
                            "(a d) e -> d a e", d=DTW), hslab[:, r, :, 0:HC])
                # r-contiguous a_src: one strided DVE gather-copy, one DMA
                nc.vector.tensor_copy(
                    out=as_sb[:].rearrange("d a (r e) -> d a r e", e=4),
                    in_=hslab[:, :, :, HC:HC + 4].rearrange(
                        "d r a e -> d a r e"))
                nc.gpsimd.dma_start(
                    h_hbm[0:N, RC * HC:RC * HC + AC].rearrange(
                        "(a d) e -> d a e", d=DTW), as_sb[:])

            # ---- edge phase (sub-chunked gathers for pipeline depth) ----
            with tc.tile_pool(name="npsum", bufs=2, space="PSUM") as npsum:
                SUBT = (maxt + 1) // 2
                for dt in range(NDT):
                    nt_ = ntiles[dt]
                    t0 = t0s[dt]
                    subs = [(0, min(SUBT, nt_))]
                    if nt_ > SUBT:
                        subs.append((SUBT, nt_))

                    # single 4-bank PSUM tile per dst-tile: num | den | eps
                    nps = npsum.tile([128, 2048], f32, tag="num")

                    # ad expand for the WHOLE dst-tile first: eps one-shot
                    # groups must close before the den group opens (PSUM
                    # zero-regions are bank-granular), and none of this
                    # depends on the gather.
                    ohTc = ohTp.tile([128, maxt, 128], bf16, tag="ohT")
                    nc.sync.dma_start(
                        ohTc[:, 0:nt_, :],
                        ohT_d[:, t0 * 128:(t0 + nt_) * 128].rearrange(
                            "p (t e) -> p t e", e=128))
                    for tg in range(nt_):
                        nc.tensor.matmul(
                            out=nps[:, EPS0 + AC * tg:EPS0 + AC * (tg + 1)],
                            lhsT=ohTc[0:DTW, tg, :],
                            rhs=hslab[:, :, dt, HC + 4:HC + 8],
                            start=True, stop=True)
                    eps_sb = edgep.tile([128, maxt, AC], f32, tag="eps")
                    nc.scalar.copy(
                        out=eps_sb[:, 0:nt_, :],
                        in_=nps[:, EPS0:EPS0 + AC * nt_].rearrange(
                            "p (t a) -> p t a", a=AC))

                    for si, (lo, hi) in enumerate(subs):
                        snt = hi - lo
                        ni = snt * 128
                        sem = gsems[dt * 2 + si]
                        hg = hgp.tile([128, SUBT, ROWW], bf16, tag="hg")
                        nc.gpsimd.dma_gather(
                            out_ap=hg[:, 0:snt, :], in_ap=h_hbm[:],
                            idxs_ap=ih[:, (t0 + lo) * 8:(t0 + hi) * 8],
                            num_idxs=ni, num_idxs_reg=ni, elem_size=ROWW,
                            single_packet=False).then_inc(sem, 16)

                        ohc = ohp.tile([128, SUBT, DTW], bf16, tag="oh")
                        nc.sync.dma_start(
                            ohc[:, 0:snt, :],
                            oh_d[:, (t0 + lo) * DTW:(t0 + hi) * DTW].rearrange(
                                "p (t d) -> p t d", d=DTW))

                        # explicit data-readiness gates for the gather
                        nc.vector.wait_ge(sem, 16)
                        nc.scalar.wait_ge(sem, 16)
                        nc.tensor.wait_ge(sem, 16)

                        # z = as + ad ; leaky relu ; exp
                        z = edgep.tile([128, SUBT, AC], f32, tag="z")
                        nc.vector.tensor_tensor(
                            out=z[:, 0:snt, :],
                            in0=hg[:, 0:snt, RC * HC:RC * HC + AC],
                            in1=eps_sb[:, lo:hi, :], op=Alu.add)
                        nc.vector.scalar_tensor_tensor(
                            out=z[:, 0:snt, :], in0=z[:, 0:snt, :],
                            scalar=NEG_SLOPE,
                            in1=z[:, 0:snt, :], op0=Alu.mult, op1=Alu.max)
                        p_bf = edgep.tile([128, SUBT, AC], bf16, tag="p")
                        nc.scalar.activation(out=p_bf[:, 0:snt, :],
                                             in_=z[:, 0:snt, :], func=Act.Exp)

                        # den = segsum(p) -- accumulates across sub-chunks
                        for t in range(snt):
                            tg = lo + t
                            nc.tensor.matmul(out=nps[0:DTW, DEN0:DEN0 + AC],
                                             lhsT=ohc[:, t, :],
                                             rhs=p_bf[:, t, :],
                                             start=(tg == 0),
                                             stop=(tg == nt_ - 1))

                        # msg = hg * p (in-place, bf16 DVE)
                        for r in range(RC):
                            hgr = hg[:, 0:snt, r * HC:(r + 1) * HC].rearrange(
                                "p t (c h) -> p t c h", h=H)
                            pb = p_bf[:, 0:snt, 4 * r:4 * r + 4].rearrange(
                                "p t (o h) -> p t o h", o=1).to_broadcast(
                                [128, snt, C, H])
                            nc.vector.tensor_tensor(out=hgr, in0=hgr, in1=pb,
                                                    op=Alu.mult)

                        # num = segsum(msg): 3x512-col accumulating matmuls
                        for t in range(snt):
                            tg = lo + t
                            for cb in range(3):
                                nc.tensor.matmul(
                                    out=nps[0:DTW, cb * 512:(cb + 1) * 512],
                                    lhsT=ohc[:, t, :],
                                    rhs=hg[:, t, cb * 512:(cb + 1) * 512],
                                    start=(tg == 0), stop=(tg == nt_ - 1))

                    drb = finp.tile([DTW, AC], f32, tag="drb")
                    nc.vector.reciprocal(out=drb[:], in_=nps[0:DTW, DEN0:DEN0 + AC])

                    # finalize: numn = num * (1/den) [single PSUM pass],
                    # head-sum, + bias, DMA out
                    numn = finp.tile([DTW, RC, C, H], bf16, tag="numn")
                    drbb = drb[:].rearrange("d (r o h) -> d r o h", h=H, o=1
                                            ).to_broadcast([DTW, RC, C, H])
                    nc.vector.tensor_tensor(
                        out=numn[:],
                        in0=nps[0:DTW, 0:RC * HC].rearrange(
                            "d (r c h) -> d r c h", h=H, c=C),
                        in1=drbb, op=Alu.mult)
                    t12 = finp.tile([DTW, RC, C], bf16, tag="t12")
                    t34 = finp.tile([DTW, RC, C], bf16, tag="t34")
                    ob = finp.tile([DTW, RC, C], f32, tag="ob")
                    nc.vector.tensor_tensor(out=t12[:], in0=numn[:, :, :, 0],
                                            in1=numn[:, :, :, 1], op=Alu.add)
                    nc.vector.tensor_tensor(out=t34[:], in0=numn[:, :, :, 2],
                                            in1=numn[:, :, :, 3], op=Alu.add)
                    nc.vector.tensor_tensor(out=ob[:], in0=t12[:], in1=t34[:],
                                            op=Alu.add)
                    nc.vector.tensor_tensor(out=ob[:], in0=ob[:],
                                            in1=bias_sl[0:DTW, :, :], op=Alu.add)
                    nc.sync.dma_start(
                        out_d[:, dt * DTW:(dt + 1) * DTW, :].rearrange(
                            "r d f -> d r f"), ob[:])

    nc.compile()
    return nc


# --------------------------------------------------------------------------
# public entry point
# --------------------------------------------------------------------------
def kernel(x, edge_index, W, att_src, att_dst, bias):
    key = hash(np.asarray(edge_index).tobytes())
    if key not in _CACHE:
        ed = _prep_edges(edge_index)
        _CACHE[key] = (_build_program(ed), ed)
    nc, ed = _CACHE[key]

    in_maps = _make_in_maps(x, W, att_src, att_dst, bias, ed)
    from concourse import bass_utils
    res = bass_utils.run_bass_kernel_spmd(nc, in_maps, core_ids=list(range(NCORES)))
    outs = [res.results[c]["out"] for c in range(NCORES)]
    out = np.concatenate(outs, axis=0).reshape(B, S, N, F).astype(np.float32)
    return out
